# revision 25
# baseline (speedup 1.0000x reference)
"""Multi-head attention forward (B=2, T=2048, C=1024, 16 heads of dim 64)
sharded 8-way tensor-parallel over heads across 8 TRN2 NeuronCores.

Each core computes 2 heads end-to-end:
  qkv^T = w_c^T @ x^T           (weight-stationary, transposed layout)
  S^T_h = k_h @ q_h^T           (zero-padded K=128 so every matmul stays in
                                 128x128 array mode -- no PE mode switches)
  P^T_h = exp(S^T_h)            (no max subtraction: scores ~N(0,1), |S|<9)
  y^T_h = [v_h | 1]^T @ P^T_h   (ones column yields softmax denominators)
  out_c = sum_h (y_h/denom) @ w_proj[head rows]   (partial projection)
Host gathers: out = sum_c out_c  (the tensor-parallel all-reduce).

All matmuls bf16 (full-rate PE); softmax stats fp32. Reciprocal runs on
DVE, the denominator row-broadcast on GpSimd, keeping ScalarE exclusively
on the bulk exp stream. Emission is software-pipelined in chunk slots:
proj(j-1) | attnV(j) | scores(j+1) | qkv(j+2..) interleaved so the PE
stream stays dense (HAM stays at 2.4 GHz).
"""

import numpy as np
import ml_dtypes
from contextlib import ExitStack

import concourse.bass as bass
import concourse.bacc as bacc
import concourse.mybir as mybir
import concourse.tile as tile
from concourse.bass_utils import run_bass_kernel_spmd
from concourse.masks import make_identity

F32 = mybir.dt.float32
BF16 = mybir.dt.bfloat16
AFT = mybir.ActivationFunctionType

P = 128
NB = 2        # batches
TB = 2048     # tokens per batch
NT = NB * TB  # 4096 tokens total
C = 1024
KC = C // P   # 8 contraction tiles for the qkv projection
QCH = 512     # q-token chunk
NCH = NT // QCH   # 8 chunks total
NQC = TB // QCH   # 4 q chunks per batch
NKT = TB // P     # 16 k tiles per batch
N_CORES = 8
HEAD_DIM = 64

# pT buffer coloring with TWO-slot score lookahead (scores for chunk j+2
# emitted in slot j, so exp(c) completes a full slot before attnV(c) needs
# it). Alive intervals: 0:[pre,0] 1:[pre,1] 2:[0,2] 3:[1,3] 4:[2,4]
# 5:[3,5] 6:[4,6] 7:[5,7] -> 3-colorable:
PT_TAG = {0: "pta", 3: "pta", 6: "pta",
          1: "ptb", 4: "ptb", 7: "ptb",
          2: "ptc", 5: "ptc"}

DEBUG_PSA = {}


def _build_program(nc: bass.Bass):
    xT = nc.declare_dram_parameter("xT", [C, NT], BF16, isOutput=False)[:]
    wqkv = nc.declare_dram_parameter("wqkv", [C, 384], BF16, isOutput=False)[:]
    wproj = nc.declare_dram_parameter("wproj", [P, C], BF16, isOutput=False)[:]
    out = nc.declare_dram_parameter("out", [NT, C], F32, isOutput=True)[:]

    with tile.TileContext(nc) as tc, ExitStack() as ctx:
        singles = ctx.enter_context(tc.tile_pool(name="singles", bufs=1))
        xpool = ctx.enter_context(tc.tile_pool(name="xpool", bufs=2))
        ptp = ctx.enter_context(tc.tile_pool(name="ptp", bufs=1))
        vtp = ctx.enter_context(tc.tile_pool(name="vtp", bufs=1))
        ybp = ctx.enter_context(tc.tile_pool(name="ybp", bufs=2))
        recp = ctx.enter_context(tc.tile_pool(name="recp", bufs=1))
        osbp = ctx.enter_context(tc.tile_pool(name="osbp", bufs=2))
        psAp = ctx.enter_context(tc.tile_pool(name="psA", bufs=2, space="PSUM"))
        psXO = ctx.enter_context(tc.tile_pool(name="psXO", bufs=2, space="PSUM"))
        psYp = ctx.enter_context(tc.tile_pool(name="psY", bufs=2, space="PSUM"))

        # ---------------- constants / persistent tensors ----------------
        # x-chunk DMAs are issued first so the first qkv matmul's inputs are
        # in flight immediately (SP issues dma_starts in emission order)
        xt_tiles = {}

        def load_x(c):
            # per-kc dma_starts spread across 8 DMA queues (parallel);
            # a single batched dma_start serializes through one queue
            t = xpool.tile([P, KC, QCH], BF16, tag="xt", name=f"xt{c}")
            csl = slice(c * QCH, (c + 1) * QCH)
            for kc in range(KC):
                nc.sync.dma_start(
                    out=t[:, kc, :], in_=xT[kc * P : (kc + 1) * P, csl]
                )
            xt_tiles[c] = t

        load_x(0)
        w_sb = singles.tile([P, KC, 384], BF16, tag="w_sb")
        nc.sync.dma_start(out=w_sb[:], in_=wqkv.rearrange("(kc p) m -> p kc m", p=P))
        load_x(1)

        wp_sb = singles.tile([P, C], BF16, tag="wp")
        nc.sync.dma_start(out=wp_sb[:], in_=wproj)

        ident = singles.tile([P, P], BF16, tag="ident")
        make_identity(nc, ident[:])

        q_sb = singles.tile([P, NT], BF16, tag="q_sb")
        # zero-padded per-head k: k0 has h0 dims on rows 0:64 (rows 64:128
        # zero), k1 has h1 dims on rows 64:128 (rows 0:64 zero). Scores then
        # run as plain K=128 matmuls with no array-mode switch.
        k0_sb = singles.tile([P, NT], BF16, tag="k0")
        k1_sb = singles.tile([P, NT], BF16, tag="k1")
        nc.vector.memset(k0_sb[HEAD_DIM:P, :], 0.0)
        nc.gpsimd.memset(k1_sb[0:HEAD_DIM, :], 0.0)

        # v_aug[:, i, h, :] = [v_h for token tile i (64 cols) | ones col]
        v_aug = singles.tile([P, NT // P, 2, HEAD_DIM + 1], BF16, tag="v_aug")
        nc.vector.memset(v_aug[:, :, :, HEAD_DIM : HEAD_DIM + 1], 1.0)

        # fmat[h] broadcasts the recip denominator (row 64 of rec) to that
        # head's 64-row block of the stacked y tile
        fmat = []
        for h in range(2):
            t = singles.tile([P, P], BF16, tag=f"fmat{h}", name=f"fmat{h}")
            nc.gpsimd.memset(t[:], 0.0)
            nc.gpsimd.memset(
                t[HEAD_DIM : HEAD_DIM + 1, h * HEAD_DIM : (h + 1) * HEAD_DIM], 1.0
            )
            fmat.append(t)

        # reciprocal staging: row 64 holds each head's 1/denom, all other
        # rows stay zero (one-time memset; only row 64 is ever rewritten)
        rec = singles.tile([P, 2, QCH], BF16, tag="rec")
        nc.vector.memset(rec[:], 0.0)

        pT = {}
        psY = {}
        yb = {}

        def emit_qkv_m(c, m):
            """One output group (m: 0=q, 1=k, 2=v) of the qkv projection for
            token chunk c, contracting over all 8 kc tiles."""
            csl = slice(c * QCH, (c + 1) * QCH)
            xt = xt_tiles[c]
            ps = psXO.tile([P, QCH], F32, tag="psxo")
            for kc in range(KC):
                nc.tensor.matmul(
                    ps[:],
                    lhsT=w_sb[:, kc, m * P : (m + 1) * P],
                    rhs=xt[:, kc, :],
                    start=(kc == 0),
                    stop=(kc == KC - 1),
                )
            if m == 0:
                nc.vector.tensor_copy(out=q_sb[:, csl], in_=ps[:])
            elif m == 1:
                nc.vector.tensor_copy(
                    out=k0_sb[0:HEAD_DIM, csl], in_=ps[0:HEAD_DIM, :]
                )
                nc.vector.tensor_copy(
                    out=k1_sb[HEAD_DIM:P, csl], in_=ps[HEAD_DIM:P, :]
                )
            else:
                vt = vtp.tile([P, QCH], BF16, tag="vt")
                nc.vector.tensor_copy(out=vt[:], in_=ps[:])
                # transpose v^T -> v via identity matmul (stays in 128x128
                # mode, unlike transpose-mode matmul)
                for j in range(QCH // P):
                    i = c * (QCH // P) + j
                    pt = psXO.tile([P, P], F32, tag="psxo")
                    nc.tensor.matmul(
                        pt[:],
                        lhsT=vt[:, j * P : (j + 1) * P],
                        rhs=ident[:],
                        start=True,
                        stop=True,
                    )
                    nc.vector.tensor_copy(
                        out=v_aug[:, i, 0, 0:HEAD_DIM], in_=pt[:, 0:HEAD_DIM]
                    )
                    nc.vector.tensor_copy(
                        out=v_aug[:, i, 1, 0:HEAD_DIM], in_=pt[:, HEAD_DIM:P]
                    )

        def emit_scw(c, w):
            """Scores + exp for one k-tile wave w (kt index within batch) of
            q-chunk c. Both heads via zero-padded K=128 matmuls."""
            if c not in pT:
                pT[c] = ptp.tile(
                    [P, NKT, 2, QCH], BF16, tag=PT_TAG[c], name=f"pT{c}"
                )
            b = c // NQC
            qsl = slice(c * QCH, (c + 1) * QCH)
            ksl = slice(b * TB + w * P, b * TB + (w + 1) * P)
            psa = psAp.tile([P, 2, QCH], F32, tag="psa")
            DEBUG_PSA[psa.tensor.name] = (c, w)
            nc.tensor.matmul(
                psa[:, 0, :], lhsT=k0_sb[:, ksl], rhs=q_sb[:, qsl],
                start=True, stop=True,
            )
            nc.tensor.matmul(
                psa[:, 1, :], lhsT=k1_sb[:, ksl], rhs=q_sb[:, qsl],
                start=True, stop=True,
            )
            nc.scalar.activation(out=pT[c][:, w, :, :], in_=psa[:], func=AFT.Exp)

        def emit_av(c, kt):
            if kt == 0:
                psY[c] = (
                    psYp.tile([P, QCH], F32, tag="psy", name=f"psY{c}h0"),
                    psYp.tile([P, QCH], F32, tag="psy", name=f"psY{c}h1"),
                )
            b = c // NQC
            for h in range(2):
                nc.tensor.matmul(
                    psY[c][h][0 : HEAD_DIM + 1, :],
                    lhsT=v_aug[:, b * NKT + kt, h, :],
                    rhs=pT[c][:, kt, h, :],
                    start=(kt == 0),
                    stop=(kt == NKT - 1),
                )

        def emit_nrm(c):
            """1/denom via Ln/Exp on ScalarE (row 64), broadcast to the
            64-row head blocks via a PE matmul, then normalize y (DVE)."""
            lnd = recp.tile([P, QCH], F32, tag="lnd")
            for h in range(2):
                nc.scalar.activation(
                    out=lnd[HEAD_DIM : HEAD_DIM + 1, :],
                    in_=psY[c][h][HEAD_DIM : HEAD_DIM + 1, :],
                    func=AFT.Ln,
                )
                nc.scalar.activation(
                    out=rec[HEAD_DIM : HEAD_DIM + 1, h, :],
                    in_=lnd[HEAD_DIM : HEAD_DIM + 1, :],
                    func=AFT.Exp,
                    scale=-1.0,
                )
            pbc = psXO.tile([P, QCH], F32, tag="psxo")
            for h in range(2):
                nc.tensor.matmul(
                    pbc[:], lhsT=fmat[h][:], rhs=rec[:, h, :],
                    start=(h == 0), stop=(h == 1),
                )
            rf = recp.tile([P, QCH], F32, tag="rf")
            nc.vector.tensor_copy(out=rf[:], in_=pbc[:])
            t = ybp.tile([P, QCH], BF16, tag="yb")
            nc.vector.tensor_mul(
                out=t[0:HEAD_DIM, :],
                in0=psY[c][0][0:HEAD_DIM, :],
                in1=rf[0:HEAD_DIM, :],
            )
            nc.vector.tensor_mul(
                out=t[HEAD_DIM:P, :],
                in0=psY[c][1][0:HEAD_DIM, :],
                in1=rf[HEAD_DIM:P, :],
            )
            yb[c] = t

        def emit_prj_tt(c, tt):
            row0 = c * QCH + tt * P
            osb = osbp.tile([P, 2, QCH], F32, tag="osb")
            for ncol in range(2):
                po = psXO.tile([P, QCH], F32, tag="psxo")
                nc.tensor.matmul(
                    po[:],
                    lhsT=yb[c][:, tt * P : (tt + 1) * P],
                    rhs=wp_sb[:, ncol * QCH : (ncol + 1) * QCH],
                    start=True,
                    stop=True,
                )
                nc.vector.tensor_copy(out=osb[:, ncol, :], in_=po[:])
            nc.sync.dma_start(out=out[row0 : row0 + P, :], in_=osb[:, :, :])

        # ---------------- emission schedule ----------------
        # Scores run TWO chunks ahead of attnV so the exp stream finishes a
        # full slot before its consumer. Batch-1 score waves are emitted
        # right after the qkv k-drain that produces their k tiles.
        #
        # prologue: batch-0 qkv with chunk-0 score waves streaming in;
        # chunk-1 scores at the end (x DMAs already issued at the top).
        for ci in range(NQC):
            for m in range(3):
                emit_qkv_m(ci, m)
            if ci + 2 < NCH:
                load_x(ci + 2)
            for w in range(4 * ci, 4 * ci + 4):
                emit_scw(0, w)
        for w in range(NKT):
            emit_scw(1, w)

        # per-slot score-wave emissions (chunk -> waves), balanced so the
        # exp stream gets a steady ~16 waves per slot, honoring k-chunk
        # availability for the batch-1 chunks (k-chunk of wave w is 4+w//4,
        # drained by qkv(4+w//4) which runs in slot w//4):
        sc_sched = {
            0: [(2, w) for w in range(NKT)],
            1: [(3, w) for w in range(NKT)],
            2: [(4, w) for w in range(12)],
            3: [(4, w) for w in range(12, NKT)] + [(5, w) for w in range(12)],
            4: [(5, w) for w in range(12, NKT)] + [(6, w) for w in range(12)],
            5: [(6, w) for w in range(12, NKT)] + [(7, w) for w in range(12)],
            6: [(7, w) for w in range(12, NKT)],
        }

        for j in range(NCH):
            avs = [("av", j, kt) for kt in range(NKT)]
            scs = [("sc", c, w) for (c, w) in sc_sched.get(j, [])]
            # score waves needing a k-chunk drained by THIS slot's qkv must
            # be emitted after it (emission order = dependency direction)
            late_scs = []
            if j <= 3:
                qkv_c = j + 4
                late_scs = [
                    u for u in scs if u[1] >= 4 and u[2] // 4 + 4 >= qkv_c
                ]
                scs = [u for u in scs if u not in late_scs]
            prjs = (
                [("prj", j - 1, tt) for tt in range(QCH // P)] if j >= 1 else []
            )
            qkv_units = (
                [("qkv", j + 4, m) for m in range(3)] if j + 4 < NCH else []
            )

            seq = []
            seq.extend(prjs[:2])
            seq.extend(qkv_units[:2])  # k drain early: frees late_scs sooner
            si = 0
            for i, u in enumerate(avs):
                seq.append(u)
                if i == 1 and qkv_units:
                    seq.append(qkv_units[2])
                if i == 2:
                    seq.extend(late_scs)
                if i == 3 and len(prjs) > 2:
                    seq.extend(prjs[2:])
                if scs:
                    take = (len(scs) * (i + 1)) // len(avs)
                    while si < take:
                        seq.append(scs[si])
                        si += 1
            seq.extend(scs[si:])

            for u in seq:
                if u[0] == "av":
                    emit_av(u[1], u[2])
                elif u[0] == "sc":
                    emit_scw(u[1], u[2])
                elif u[0] == "prj":
                    emit_prj_tt(u[1], u[2])
                else:
                    _, cc, m = u
                    if m == 0 and cc + 2 < NCH:
                        load_x(cc + 2)
                    emit_qkv_m(cc, m)
            emit_nrm(j)

        # epilogue: last chunk's projection
        for tt in range(QCH // P):
            emit_prj_tt(NCH - 1, tt)
    return nc


def _prepare_in_maps(x, w_attn, w_proj):
    bf16 = ml_dtypes.bfloat16
    x = np.asarray(x, dtype=np.float32)
    w_attn = np.asarray(w_attn, dtype=np.float32)
    w_proj = np.asarray(w_proj, dtype=np.float32)

    xT = np.ascontiguousarray(x.reshape(NT, C).T.astype(bf16))  # [C, NT]
    in_maps = []
    for c in range(N_CORES):
        h0, h1 = 2 * c, 2 * c + 1
        cols = []
        for h in (h0, h1):  # q columns, pre-scaled by softmax 1/sqrt(64)
            cols.append(w_attn[:, h * HEAD_DIM : (h + 1) * HEAD_DIM] * 0.125)
        for h in (h0, h1):  # k columns
            cols.append(w_attn[:, C + h * HEAD_DIM : C + (h + 1) * HEAD_DIM])
        for h in (h0, h1):  # v columns
            cols.append(w_attn[:, 2 * C + h * HEAD_DIM : 2 * C + (h + 1) * HEAD_DIM])
        wqkv_c = np.ascontiguousarray(np.concatenate(cols, axis=1).astype(bf16))
        wproj_c = np.ascontiguousarray(
            np.concatenate(
                [
                    w_proj[h0 * HEAD_DIM : (h0 + 1) * HEAD_DIM, :],
                    w_proj[h1 * HEAD_DIM : (h1 + 1) * HEAD_DIM, :],
                ]
            ).astype(bf16)
        )  # [128, C]
        in_maps.append({"xT": xT, "wqkv": wqkv_c, "wproj": wproj_c})
    return in_maps


class _AttnBacc(bacc.Bacc):
    """Pin all activations to natural_log_exp_and_others so the exp stream
    issues a single ACT table load."""

    def insert_act_table_loads(self):
        import bass_rust as _bass_rust
        from concourse.hw_specs import get_activation_tables

        has_activation = any(
            isinstance(i, mybir.InstActivation)
            for b in self.main_func.blocks
            for i in b.instructions
        )
        if not has_activation:
            return
        tables = []
        for name, fns in get_activation_tables(self.m.arch).items():
            if name != "natural_log_exp_and_others":
                fns = set()
            tables.append((name, fns))
        _bass_rust.insert_act_table_loads(self, tables)


_CACHED_NC = None


def _get_nc():
    global _CACHED_NC
    if _CACHED_NC is None:
        _CACHED_NC = _build_program(_AttnBacc())
        _CACHED_NC.finalize()
    return _CACHED_NC


def run(x, w_attn, w_proj, trace=False):
    """Returns (output [B, TB, C] float32, BassKernelResults)."""
    in_maps = _prepare_in_maps(x, w_attn, w_proj)
    nc = _get_nc()
    res = run_bass_kernel_spmd(nc, in_maps, core_ids=list(range(N_CORES)), trace=trace)
    acc = np.zeros((NT, C), dtype=np.float64)
    for r in res.results:
        acc += r["out"].astype(np.float64)
    return acc.astype(np.float32).reshape(NB, TB, C), res


def kernel(x, w_attn, w_proj):
    out, _ = run(x, w_attn, w_proj, trace=False)
    return out
